# revision 9
# baseline (speedup 1.0000x reference)
"""Differential Transformer kernel for TRN2, 8 cores.

Sharding: 2 batch groups x 4-way tensor parallel over heads. Core c = (group
g=c//4, rank r=c%4) handles batch g, heads [3r, 3r+3), and ALL 2048 rows of
attention for those heads. K/Q/V stay SBUF-resident (no K/V gathers). After
the Wo partial product a chunked ReduceScatter (4x [512,1536]->[128,1536])
sums head contributions and scatters rows; each core then owns the fixed
interleaved row set I_r = {cc*512 + r*128 + i} on which it runs the residual
add, ln2, and the (row-local, full-weight) SwiGLU FFN. Between layers one
bf16 AllGather of the rms'd hidden state (chan-major [1536,512] blocks)
rebuilds the full h^T; the un-permuting read-back keeps both layers in
natural row order with a fixed owned set. Final Wout is row-local.

Precision: h/Q/K/V/scores inputs bf16 (psum f32), group-norm + residual +
ln2 + w1/w3 in f32(r), attention E/V + Wo + w2 + Wout in bf16.

SBUF tags are overlaid across phases (same tag = same reserved slot):
  hT48: hT [12,2048]bf16 / fT [32,512]bf16
  kq16: kqh [4,2048]bf16 / w2h [16,512]bf16
  o24:  oT [6,2048]bf16 / h2T [12,512]f32 / hnT [12,512]bf16
  sh24: vt [16,771]bf16 / ynat [4,1536]f32
  wo18: wocN [6,1536]bf16 / xbf [12,512]bf16
  resid: xres == x2nat (in-place rotate)
"""

from contextlib import ExitStack
from dataclasses import dataclass

import numpy as np

import concourse.bass as bass
import concourse.mybir as mybir
import concourse.tile as tile
from concourse.masks import make_identity

F32 = mybir.dt.float32
F32R = mybir.dt.float32r
BF16 = mybir.dt.bfloat16
AF = mybir.ActivationFunctionType
ALU = mybir.AluOpType


@dataclass
class Cfg:
    R: int = 512          # owned rows per core
    RALL: int = 2048      # full sequence rows (= keys)
    D: int = 1536         # model dim
    H: int = 12           # total heads
    HL: int = 3           # heads per core
    HFF: int = 4096       # ffn hidden
    V: int = 32000        # vocab
    DEPTH: int = 2
    NG: int = 4           # cores per batch group
    EPS: float = 1e-6
    LAM_INIT: float = float(0.8 - 0.6 * np.exp(-0.3 * 2))

    @property
    def HD(self):
        return self.D // self.H     # 128

    @property
    def DT(self):
        return self.D // 128        # 12

    @property
    def CT(self):
        return 2 * self.HL          # local qkv chan tiles (6 x 128)

    @property
    def CB(self):
        return self.RALL // 512     # 512-col chunks of full rows (4)

    @property
    def KC(self):
        return self.RALL // 128     # key chunks (16)

    @property
    def RT(self):
        return self.R // 128        # owned row tiles (4)

    @property
    def HFT(self):
        return self.HFF // 128      # 32


def _vchunks(V):
    out = []
    off = 0
    while off < V:
        out.append((off, min(512, V - off)))
        off += 512
    return out


def r_(ap):
    return ap.bitcast(F32R)


def build_kernel(tc: tile.TileContext, ins: dict, outs: dict, cfg: Cfg,
                 replica_groups):
    nc = tc.nc
    c = cfg

    assert c.HD == 128
    scale = c.HD ** -0.5
    MO = c.D // 512  # 3

    ctx = ExitStack()
    with ctx:
        bigp = ctx.enter_context(tc.tile_pool(name="bigp", bufs=1))
        eh_p = ctx.enter_context(tc.tile_pool(name="eh", bufs=2))
        osb_p = ctx.enter_context(tc.tile_pool(name="osb", bufs=1))
        scr_p = ctx.enter_context(tc.tile_pool(name="scr", bufs=2))
        st_p = ctx.enter_context(tc.tile_pool(name="st", bufs=4))
        v1_p = ctx.enter_context(tc.tile_pool(name="v1", bufs=2))
        w_p = ctx.enter_context(tc.tile_pool(name="wp", bufs=2))
        misc_p = ctx.enter_context(tc.tile_pool(name="misc", bufs=1))
        ps = ctx.enter_context(tc.tile_pool(name="ps", bufs=1, space="PSUM"))
        psm = ctx.enter_context(tc.tile_pool(name="psm", bufs=2, space="PSUM"))
        pss = ctx.enter_context(tc.tile_pool(name="pss", bufs=2, space="PSUM"))
        dram = ctx.enter_context(tc.tile_pool(name="dram", bufs=2, space="DRAM"))

        # constants
        ident = misc_p.tile([128, 128], F32, tag="ident")
        make_identity(nc, ident)
        ones1f = misc_p.tile([1, 128], F32, tag="ones1f")
        nc.vector.memset(ones1f, 1.0)
        ones1 = misc_p.tile([1, 128], F32R, tag="ones1")
        nc.vector.tensor_copy(ones1, ones1f)
        ones128f = misc_p.tile([128, 1], F32, tag="ones128f")
        nc.vector.memset(ones128f, 1.0)
        ones128 = misc_p.tile([128, 1], F32R, tag="ones128")
        nc.vector.tensor_copy(ones128, ones128f)
        eps1 = misc_p.tile([1, 1], F32, tag="eps1")
        nc.vector.memset(eps1, c.EPS)
        eps128 = misc_p.tile([128, 1], F32, tag="eps128")
        nc.vector.memset(eps128, c.EPS)
        nlam = misc_p.tile([128, c.DEPTH * c.HL], F32, tag="nlam")
        nc.sync.dma_start(out=nlam,
                          in_=ins["neglam"].to_broadcast((128, c.DEPTH * c.HL)))

        # residual rows (owned set I_r), natural [row-part, cc-block, chan]
        xres = bigp.tile([128, c.RT, c.D], F32, tag="resid")
        nc.sync.dma_start(out=xres,
                          in_=ins["xres"].rearrange("(b p) d -> p b d", p=128))

        x2nat = None
        ag_out = None

        for layer in range(c.DEPTH):
            wq, wk, wv = ins[f"wq{layer}"], ins[f"wk{layer}"], ins[f"wv{layer}"]

            # ---- h^T full [128, DT, RALL] bf16 ----
            hT = bigp.tile([128, c.DT, c.RALL], BF16, tag="hT48")
            if layer == 0:
                # rms over channels, two-pass streamed from the x^T input
                for cb in range(c.CB):
                    cs = slice(cb * 512, (cb + 1) * 512)
                    xcb = bigp.tile([128, c.DT, 512], F32, tag="o24")
                    nc.sync.dma_start(
                        out=xcb,
                        in_=ins["xT"][:, cs].rearrange("(t p) r -> p t r", p=128))
                    ssq = psm.tile([1, 512], F32, tag="mm")
                    for t in range(c.DT):
                        sqv = scr_p.tile([128, 512], F32R, tag="g512")
                        nc.vector.tensor_mul(sqv, r_(xcb[:, t, :]), r_(xcb[:, t, :]))
                        nc.tensor.matmul(ssq, ones128, sqv,
                                         start=(t == 0), stop=(t == c.DT - 1),
                                         skip_group_check=True)
                    sq_sb = v1_p.tile([1, 512], F32, tag="v1")
                    nc.scalar.activation(sq_sb, ssq, AF.Sqrt, bias=eps1,
                                         scale=1.0 / c.D)
                    rs = v1_p.tile([1, 512], F32R, tag="v1")
                    with nc.allow_low_precision(reason="f32r rms scale bcast"):
                        nc.vector.reciprocal(rs, sq_sb)
                    bc_ps = psm.tile([128, 512], F32, tag="mm")
                    nc.tensor.matmul(bc_ps, ones1, rs, start=True, stop=True)
                    bc = scr_p.tile([128, 512], F32, tag="g512")
                    nc.vector.tensor_copy(bc, bc_ps)
                    for t in range(c.DT):
                        nc.vector.tensor_mul(hT[:, t, cs], xcb[:, t, :], bc)
            # (layer>0: hT filled per AG chunk inside the proj loop below)

            # ---- V natural with ones cols: [128, KC, HL*257] bf16 ----
            vt = bigp.tile([128, c.KC, c.HL * 257], BF16, tag="sh24")
            for h in range(c.HL):
                nc.vector.memset(vt[:, :, h * 257 + 256:h * 257 + 257], 1.0)
            rt_halves = ([range(c.KC)] if layer == 0
                         else [range(0, 8), range(8, 16)])
            kqh0 = None
            for jh, rts in enumerate(rt_halves):
                if layer > 0:
                    # un-permuting read-back of AG chunk jh (chan-major blocks)
                    for rk in range(c.NG):
                        for t in range(c.DT):
                            for cj in range(2):
                                cc = 2 * jh + cj
                                nc.sync.dma_start(
                                    out=hT[:, t, cc * 512 + rk * 128:
                                           cc * 512 + (rk + 1) * 128],
                                    in_=ag_out[jh][rk * c.D + t * 128:
                                                   rk * c.D + (t + 1) * 128,
                                                   cj * 128:(cj + 1) * 128])
                for h in range(c.HL):
                    wvc = w_p.tile([128, c.DT, 256], BF16, tag="wb6")
                    nc.sync.dma_start(
                        out=wvc,
                        in_=wv[:, h * 256:(h + 1) * 256]
                        .rearrange("(t p) f -> p t f", p=128))
                    for rt in rts:
                        vps = psm.tile([128, 256], F32, tag="mm")
                        for t in range(c.DT):
                            nc.tensor.matmul(
                                vps, hT[:, t, rt * 128:(rt + 1) * 128],
                                wvc[:, t, :],
                                start=(t == 0), stop=(t == c.DT - 1))
                        nc.vector.tensor_copy(vt[:, rt, h * 257:h * 257 + 256],
                                              vps)
                if layer > 0:
                    # wave-0 K/Q proj for this chunk's column half, issued
                    # early so PE has work while the other AG chunk lands
                    if kqh0 is None:
                        kqh0 = bigp.tile([128, 4, c.RALL], BF16, tag="kq16")
                    for a in range(2):
                        for wi, wsrc in enumerate((wk, wq)):
                            wc = w_p.tile([128, c.DT, 128], BF16, tag="wsmb")
                            nc.sync.dma_start(
                                out=wc,
                                in_=wsrc[:, a * 128:(a + 1) * 128]
                                .rearrange("(t p) f -> p t f", p=128))
                            for cb in (range(2) if jh == 0 else range(2, 4)):
                                cs = slice(cb * 512, (cb + 1) * 512)
                                pp = psm.tile([128, 512], F32, tag="mm")
                                for t in range(c.DT):
                                    nc.tensor.matmul(pp, wc[:, t, :],
                                                     hT[:, t, cs],
                                                     start=(t == 0),
                                                     stop=(t == c.DT - 1))
                                nc.scalar.copy(kqh0[:, 2 * wi + a, cs], pp)

            # ---- per-head waves: K/Q proj + scores + attn@V + epilogue ----
            oT = bigp.tile([128, c.CT, c.RALL], BF16, tag="o24")
            for h in range(c.HL):
                if h == 0 and kqh0 is not None:
                    kqh = kqh0
                else:
                    kqh = bigp.tile([128, 4, c.RALL], BF16, tag="kq16")
                    for a in range(2):
                        for wi, wsrc in enumerate((wk, wq)):
                            wc = w_p.tile([128, c.DT, 128], BF16, tag="wsmb")
                            nc.sync.dma_start(
                                out=wc,
                                in_=wsrc[:, h * 256 + a * 128:
                                         h * 256 + (a + 1) * 128]
                                .rearrange("(t p) f -> p t f", p=128))
                            for cb in range(c.CB):
                                cs = slice(cb * 512, (cb + 1) * 512)
                                pp = psm.tile([128, 512], F32, tag="mm")
                                for t in range(c.DT):
                                    nc.tensor.matmul(pp, wc[:, t, :],
                                                     hT[:, t, cs],
                                                     start=(t == 0),
                                                     stop=(t == c.DT - 1))
                                nc.scalar.copy(kqh[:, 2 * wi + a, cs], pp)
                for qb in range(c.CB):
                    qs = slice(qb * 512, (qb + 1) * 512)
                    o1sb = osb_p.tile([128, c.RT, 257], BF16, tag="osb")
                    for a in range(2):
                        ops = ps.tile([128, c.RT, 512], F32, tag="oacc")
                        for kc in range(c.KC):
                            sps = pss.tile([128, 512], F32, tag="sc")
                            nc.tensor.matmul(
                                sps, kqh[:, a, kc * 128:(kc + 1) * 128],
                                kqh[:, 2 + a, qs], start=True, stop=True)
                            ee = eh_p.tile([128, 512], BF16, tag="eh")
                            nc.scalar.activation(ee, sps, AF.Exp, scale=scale)
                            for rt in range(c.RT):
                                nc.tensor.matmul(
                                    ops[:, rt, 0:257],
                                    ee[:, rt * 128:(rt + 1) * 128],
                                    vt[:, kc, h * 257:(h + 1) * 257],
                                    start=(kc == 0), stop=(kc == c.KC - 1),
                                    skip_group_check=True)
                        if a == 0:
                            for rt in range(c.RT):
                                nc.scalar.copy(o1sb[:, rt, :], ops[:, rt, 0:257])
                        else:
                            for rt in range(c.RT):
                                r1 = st_p.tile([128, 1], F32, tag="r1")
                                nc.vector.reciprocal(r1, o1sb[:, rt, 256:257])
                                r2 = st_p.tile([128, 1], F32, tag="r2")
                                nc.vector.reciprocal(r2, ops[:, rt, 256:257])
                                sc2 = st_p.tile([128, 1], F32, tag="sc2")
                                nc.vector.tensor_mul(
                                    sc2, r2,
                                    nlam[:, layer * c.HL + h:layer * c.HL + h + 1])
                                t1 = scr_p.tile([128, 256], F32, tag="s256")
                                nc.vector.tensor_scalar_mul(t1, o1sb[:, rt, 0:256], r1)
                                oc = scr_p.tile([128, 256], F32, tag="s256")
                                nc.vector.scalar_tensor_tensor(
                                    out=oc, in0=ops[:, rt, 0:256], scalar=sc2,
                                    in1=t1, op0=ALU.mult, op1=ALU.add)
                                ssq = st_p.tile([128, 1], F32, tag="ssq")
                                sqo = scr_p.tile([128, 256], F32, tag="s256")
                                nc.vector.tensor_mul(sqo, oc, oc)
                                nc.vector.reduce_sum(ssq, sqo,
                                                     axis=mybir.AxisListType.X)
                                sqr = st_p.tile([128, 1], F32, tag="sqr")
                                nc.scalar.activation(sqr, ssq, AF.Sqrt,
                                                     bias=eps128, scale=1.0 / 256)
                                rinv = st_p.tile([128, 1], F32, tag="rinv")
                                nc.vector.reciprocal(rinv, sqr)
                                onr = scr_p.tile([128, 256], F32, tag="s256")
                                nc.vector.tensor_scalar_mul(onr, oc, rinv)
                                for half in range(2):
                                    tp = pss.tile([128, 128], F32, tag="sc")
                                    nc.tensor.transpose(
                                        tp, onr[:, half * 128:(half + 1) * 128],
                                        ident)
                                    nc.vector.tensor_copy(
                                        oT[:, 2 * h + half,
                                           qb * 512 + rt * 128:
                                           qb * 512 + (rt + 1) * 128], tp)

            # ---- Wo (natural out) + chunked ReduceScatter ----
            wocN = bigp.tile([128, c.CT, c.D], BF16, tag="wo18")
            nc.sync.dma_start(
                out=wocN,
                in_=ins[f"wo{layer}"].rearrange("(t p) f -> p t f", p=128))
            rs_outs = []
            for qb in range(c.CB):
                rs_in = dram.tile([512, c.D], F32, tag="rs_in")
                for rtb in range(4):
                    for mo in range(MO):
                        yps = psm.tile([128, 512], F32, tag="mm")
                        for ft in range(c.CT):
                            nc.tensor.matmul(
                                yps,
                                oT[:, ft, qb * 512 + rtb * 128:
                                   qb * 512 + (rtb + 1) * 128],
                                wocN[:, ft, mo * 512:(mo + 1) * 512],
                                start=(ft == 0), stop=(ft == c.CT - 1))
                        ystg = scr_p.tile([128, 512], F32, tag="sqy")
                        nc.scalar.copy(ystg, yps)
                        nc.sync.dma_start(
                            out=rs_in[rtb * 128:(rtb + 1) * 128,
                                      mo * 512:(mo + 1) * 512],
                            in_=ystg)
                rs_out = dram.tile([128, c.D], F32, tag=f"rs_out{qb}")
                nc.gpsimd.collective_compute(
                    "ReduceScatter", ALU.add, replica_groups=replica_groups,
                    ins=[rs_in.opt()], outs=[rs_out.opt()])
                rs_outs.append(rs_out)

            # ---- y rows (owned) + ln2 + transpose to h2T ----
            ynat = bigp.tile([128, c.RT, c.D], F32, tag="sh24")
            h2T = bigp.tile([128, c.DT, c.R], F32R, tag="o24")
            for cc in range(c.RT):
                for mo in range(MO):
                    yblk = scr_p.tile([128, 512], F32, tag="sqy")
                    nc.sync.dma_start(
                        out=yblk, in_=rs_outs[cc][:, mo * 512:(mo + 1) * 512])
                    nc.vector.tensor_add(
                        ynat[:, cc, mo * 512:(mo + 1) * 512], yblk,
                        xres[:, cc, mo * 512:(mo + 1) * 512])
                ssy = st_p.tile([128, 1], F32, tag="ssy")
                for mo in range(MO):
                    sqy = scr_p.tile([128, 512], F32, tag="sqy")
                    nc.vector.tensor_mul(
                        sqy, ynat[:, cc, mo * 512:(mo + 1) * 512],
                        ynat[:, cc, mo * 512:(mo + 1) * 512])
                    smo = st_p.tile([128, 1], F32, tag=f"smo{mo}")
                    nc.vector.reduce_sum(smo, sqy, axis=mybir.AxisListType.X)
                    if mo == 0:
                        nc.vector.tensor_copy(ssy, smo)
                    else:
                        nc.vector.tensor_add(ssy, ssy, smo)
                sqr = st_p.tile([128, 1], F32, tag="sqr")
                nc.scalar.activation(sqr, ssy, AF.Sqrt, bias=eps128,
                                     scale=1.0 / c.D)
                rinv = st_p.tile([128, 1], F32, tag="rinv")
                nc.vector.reciprocal(rinv, sqr)
                for t in range(c.DT):
                    h2n = scr_p.tile([128, 128], F32, tag="h2n")
                    nc.vector.tensor_scalar_mul(
                        h2n, ynat[:, cc, t * 128:(t + 1) * 128], rinv)
                    tp = pss.tile([128, 128], F32, tag="sc")
                    nc.tensor.transpose(tp, h2n, ident)
                    nc.vector.tensor_copy(
                        h2T[:, t, cc * 128:(cc + 1) * 128], tp)

            # ---- FFN on owned rows (full weights), natural out ----
            fT = bigp.tile([128, c.HFT, c.R], BF16, tag="hT48")
            for m in range(c.HFT):
                w1c = w_p.tile([128, c.DT, 128], F32R, tag="wsm")
                nc.sync.dma_start(
                    out=w1c,
                    in_=ins["w1"][:, m * 128:(m + 1) * 128]
                    .rearrange("(t p) f -> p t f", p=128))
                w3c = w_p.tile([128, c.DT, 128], F32R, tag="wsm")
                nc.sync.dma_start(
                    out=w3c,
                    in_=ins["w3"][:, m * 128:(m + 1) * 128]
                    .rearrange("(t p) f -> p t f", p=128))
                gps = psm.tile([128, 512], F32, tag="mm")
                for t in range(c.DT):
                    nc.tensor.matmul(gps, w1c[:, t, :], h2T[:, t, :],
                                     start=(t == 0), stop=(t == c.DT - 1))
                ups = pss.tile([128, 512], F32, tag="sc")
                for t in range(c.DT):
                    nc.tensor.matmul(ups, w3c[:, t, :], h2T[:, t, :],
                                     start=(t == 0), stop=(t == c.DT - 1))
                gsg = scr_p.tile([128, c.R], F32, tag="g512")
                nc.scalar.activation(gsg, gps, AF.Sigmoid)
                gsil = scr_p.tile([128, c.R], F32, tag="g512")
                nc.vector.tensor_mul(gsil, gsg, gps)
                nc.vector.tensor_mul(fT[:, m, :], gsil, ups)
            x2nat = bigp.tile([128, c.RT, c.D], F32, tag="resid")
            HH = c.HFT // 2
            for mo in range(MO):
                for half in range(2):
                    w2h = bigp.tile([128, HH, 512], BF16, tag="kq16")
                    nc.sync.dma_start(
                        out=w2h,
                        in_=ins["w2"][half * HH * 128:(half + 1) * HH * 128,
                                      mo * 512:(mo + 1) * 512]
                        .rearrange("(t p) f -> p t f", p=128))
                    for rb in range(c.RT):
                        xps = psm.tile([128, 512], F32, tag="mm")
                        for m in range(HH):
                            nc.tensor.matmul(
                                xps,
                                fT[:, half * HH + m, rb * 128:(rb + 1) * 128],
                                w2h[:, m, :],
                                start=(m == 0), stop=(m == HH - 1),
                                skip_group_check=True)
                        if half == 0:
                            nc.vector.tensor_add(
                                x2nat[:, rb, mo * 512:(mo + 1) * 512], xps,
                                ynat[:, rb, mo * 512:(mo + 1) * 512])
                        else:
                            nc.vector.tensor_add(
                                x2nat[:, rb, mo * 512:(mo + 1) * 512], xps,
                                x2nat[:, rb, mo * 512:(mo + 1) * 512])

            if layer < c.DEPTH - 1:
                # ---- h_next = rms(x2) rows, transpose, stage, AllGather ----
                hnT = bigp.tile([128, c.DT, c.R], BF16, tag="o24")
                for cc in range(c.RT):
                    ssy = st_p.tile([128, 1], F32, tag="ssy")
                    for mo in range(MO):
                        sqy = scr_p.tile([128, 512], F32, tag="sqy")
                        nc.vector.tensor_mul(
                            sqy, x2nat[:, cc, mo * 512:(mo + 1) * 512],
                            x2nat[:, cc, mo * 512:(mo + 1) * 512])
                        smo = st_p.tile([128, 1], F32, tag=f"smo{mo}")
                        nc.vector.reduce_sum(smo, sqy, axis=mybir.AxisListType.X)
                        if mo == 0:
                            nc.vector.tensor_copy(ssy, smo)
                        else:
                            nc.vector.tensor_add(ssy, ssy, smo)
                    sqr = st_p.tile([128, 1], F32, tag="sqr")
                    nc.scalar.activation(sqr, ssy, AF.Sqrt, bias=eps128,
                                         scale=1.0 / c.D)
                    rinv = st_p.tile([128, 1], F32, tag="rinv")
                    nc.vector.reciprocal(rinv, sqr)
                    for t in range(c.DT):
                        h2n = scr_p.tile([128, 128], F32, tag="h2n")
                        nc.vector.tensor_scalar_mul(
                            h2n, x2nat[:, cc, t * 128:(t + 1) * 128], rinv)
                        tp = pss.tile([128, 128], F32, tag="sc")
                        nc.tensor.transpose(tp, h2n, ident)
                        nc.vector.tensor_copy(
                            hnT[:, t, cc * 128:(cc + 1) * 128], tp)
                ag_outs = []
                for j in range(2):
                    js = slice(j * 256, (j + 1) * 256)
                    ag_in = dram.tile([c.D, 256], BF16, tag=f"ag_in{j}")
                    nc.sync.dma_start(
                        out=ag_in.rearrange("(t p) r -> p t r", p=128),
                        in_=hnT[:, :, js])
                    ag_out = dram.tile([c.NG * c.D, 256], BF16, tag=f"ag_out{j}")
                    nc.gpsimd.collective_compute(
                        "AllGather", ALU.bypass, replica_groups=replica_groups,
                        ins=[ag_in.opt()], outs=[ag_out.opt()])
                    ag_outs.append(ag_out)
                ag_out = ag_outs
            # owned residual for next layer
            xres = x2nat

        # ---- final projection: out = x2 @ Wout + bout (owned rows) ----
        xbf = bigp.tile([128, c.DT, c.R], BF16, tag="wo18")
        for cc in range(c.RT):
            for t in range(c.DT):
                tp = pss.tile([128, 128], F32, tag="sc")
                nc.tensor.transpose(
                    tp, x2nat[:, cc, t * 128:(t + 1) * 128], ident)
                nc.vector.tensor_copy(xbf[:, t, cc * 128:(cc + 1) * 128], tp)
        out_d = outs["out"]
        DTH = c.DT // 2
        for ci, (voff, vn) in enumerate(_vchunks(c.V)):
            woucs = []
            for hf in range(2):
                wouc = w_p.tile([128, c.DT // 2, 512], BF16, tag="wb6")
                nc.sync.dma_start(
                    out=wouc[:, :, 0:vn],
                    in_=ins["wout"][hf * DTH * 128:(hf + 1) * DTH * 128,
                                    voff:voff + vn]
                    .rearrange("(t p) f -> p t f", p=128))
                woucs.append(wouc)
            bch = v1_p.tile([1, 512], F32R, tag="v1")
            nc.sync.dma_start(out=bch[:, 0:vn], in_=ins["bout"][:, voff:voff + vn])
            for rt in range(c.RT):
                op = psm.tile([128, 512], F32, tag="mm")
                for t in range(c.DT):
                    nc.tensor.matmul(op[:, 0:vn],
                                     xbf[:, t, rt * 128:(rt + 1) * 128],
                                     woucs[t // DTH][:, t % DTH, 0:vn],
                                     start=(t == 0), stop=False,
                                     skip_group_check=True)
                nc.tensor.matmul(op[:, 0:vn], ones1, bch[:, 0:vn],
                                 start=False, stop=True, skip_group_check=True)
                ost = scr_p.tile([128, 512], F32, tag="sqy")
                if (ci + rt) % 2 == 0:
                    nc.vector.tensor_copy(ost[:, 0:vn], op[:, 0:vn])
                else:
                    nc.scalar.copy(ost[:, 0:vn], op[:, 0:vn])
                nc.sync.dma_start(
                    out=out_d[rt * 128:(rt + 1) * 128, voff:voff + vn],
                    in_=ost[:, 0:vn])


def host_inputs(cfg: Cfg, core: int, x, Wq, Wk, Wv, lq1, lq2, lk1, lk2, Wo,
                w1, w2, w3, Wout, bout):
    """Build the per-core input map (numpy) from full fp32 inputs."""
    import ml_dtypes
    c = cfg
    g, r = divmod(core, c.NG)
    hs = slice(r * c.HL, (r + 1) * c.HL)
    lam = (np.exp(np.sum(lq1 * lk1, -1)) + np.exp(np.sum(lq2 * lk2, -1))
           + c.LAM_INIT)  # [DEPTH, H]
    # owned interleaved rows I_r = {cc*512 + r*128 + i}
    own = np.concatenate([np.arange(cc * 512 + r * 128, cc * 512 + r * 128 + 128)
                          for cc in range(c.RT)])
    inm = {
        "xT": np.ascontiguousarray(x[g].T).astype(np.float32),
        "xres": np.ascontiguousarray(x[g][own, :]).astype(np.float32),
        "neglam": np.ascontiguousarray(-lam[:, hs].reshape(1, -1)).astype(np.float32),
        "w1": np.ascontiguousarray(w1).astype(np.float32),
        "w2": np.ascontiguousarray(w2).astype(ml_dtypes.bfloat16),
        "w3": np.ascontiguousarray(w3).astype(np.float32),
        "wout": np.ascontiguousarray(Wout).astype(ml_dtypes.bfloat16),
        "bout": np.ascontiguousarray(bout.reshape(1, -1)).astype(np.float32),
    }
    for l in range(c.DEPTH):
        inm[f"wq{l}"] = np.ascontiguousarray(
            Wq[l, hs].transpose(1, 0, 2).reshape(c.D, 2 * c.HD * c.HL)
        ).astype(ml_dtypes.bfloat16)
        inm[f"wk{l}"] = np.ascontiguousarray(
            Wk[l, hs].transpose(1, 0, 2).reshape(c.D, 2 * c.HD * c.HL)
        ).astype(ml_dtypes.bfloat16)
        inm[f"wv{l}"] = np.ascontiguousarray(
            Wv[l, hs].transpose(1, 0, 2).reshape(c.D, 2 * c.HD * c.HL)
        ).astype(ml_dtypes.bfloat16)
        inm[f"wo{l}"] = np.ascontiguousarray(
            Wo[l][r * c.HL * 2 * c.HD:(r + 1) * c.HL * 2 * c.HD, :]
            * (1.0 - c.LAM_INIT)).astype(ml_dtypes.bfloat16)
    return inm


def input_specs(cfg: Cfg):
    c = cfg
    CW = 2 * c.HD * c.HL  # 768 local qkv chans
    sp = {
        "xT": ([c.D, c.RALL], F32),
        "xres": ([c.R, c.D], F32),
        "neglam": ([1, c.DEPTH * c.HL], F32),
        "w1": ([c.D, c.HFF], F32R),
        "w2": ([c.HFF, c.D], BF16),
        "w3": ([c.D, c.HFF], F32R),
        "wout": ([c.D, c.V], BF16),
        "bout": ([1, c.V], F32R),
    }
    for l in range(c.DEPTH):
        sp[f"wq{l}"] = ([c.D, CW], BF16)
        sp[f"wk{l}"] = ([c.D, CW], BF16)
        sp[f"wv{l}"] = ([c.D, CW], BF16)
        sp[f"wo{l}"] = ([CW, c.D], BF16)
    return sp


# ======================================================================
# Harness entry point: kernel(**inputs) -> full output [2, 2048, 32000]
# ======================================================================

_BUILT = {}


def _build_nc():
    from concourse import bacc
    cfg = Cfg()
    rg = [[0, 1, 2, 3], [4, 5, 6, 7]]
    nc = bacc.Bacc("TRN2", target_bir_lowering=False, debug=False,
                   num_devices=8)
    ins_ap, outs_ap = {}, {}
    for name, (shape, dt) in input_specs(cfg).items():
        ins_ap[name] = nc.dram_tensor(name, shape, dt,
                                      kind="ExternalInput").ap()
    outs_ap["out"] = nc.dram_tensor("out", [cfg.R, cfg.V], mybir.dt.float32,
                                    kind="ExternalOutput").ap()
    with tile.TileContext(nc) as tc:
        build_kernel(tc, ins_ap, outs_ap, cfg, rg)
    nc.compile()
    return cfg, nc


def kernel(x, Wq, Wk, Wv, lq1, lq2, lk1, lk2, Wo, w1, w2, w3, Wout, bout):
    from concourse.bass_utils import run_bass_kernel_spmd
    if "nc" not in _BUILT:
        _BUILT["cfg"], _BUILT["nc"] = _build_nc()
    cfg, nc = _BUILT["cfg"], _BUILT["nc"]
    args = dict(x=np.asarray(x, np.float32), Wq=np.asarray(Wq, np.float32),
                Wk=np.asarray(Wk, np.float32), Wv=np.asarray(Wv, np.float32),
                lq1=np.asarray(lq1, np.float32), lq2=np.asarray(lq2, np.float32),
                lk1=np.asarray(lk1, np.float32), lk2=np.asarray(lk2, np.float32),
                Wo=np.asarray(Wo, np.float32), w1=np.asarray(w1, np.float32),
                w2=np.asarray(w2, np.float32), w3=np.asarray(w3, np.float32),
                Wout=np.asarray(Wout, np.float32),
                bout=np.asarray(bout, np.float32))
    in_maps = [host_inputs(cfg, core, **args) for core in range(8)]
    r = run_bass_kernel_spmd(nc, in_maps, core_ids=list(range(8)))
    B, S = 2, cfg.RALL
    out = np.empty((B, S, cfg.V), np.float32)
    for core in range(8):
        g, rk = divmod(core, cfg.NG)
        for cc in range(cfg.RT):
            out[g, cc * 512 + rk * 128:cc * 512 + rk * 128 + 128, :] = \
                r.results[core]["out"][cc * 128:(cc + 1) * 128]
    return out
